# revision 1
# baseline (speedup 1.0000x reference)
"""Trainium2 Bass kernel for nn_CombinatorialClassifier.

Computation (reference):
    logits = einsum('bf,pqf->bpq', x, W) + b        # [B,P,Q]
    logp   = log_softmax(logits, axis=2)            # [B,P,Q]
    out    = take_along_axis(logp, part_idx, 2)     # [B,P,C]

Shapes: B=256, P=64, Q=128, C=1000, F=2048.

Sharding: expert-parallel over P across 8 cores (8 partitionings per
core).  Each core reads the full x and its W/b/part_idx slice and
writes its disjoint [B, 8, C] slice of the output.  No collectives.

Per-core dataflow ("orientation A" — q lives on SBUF partitions):
  - main matmul:   psum_lin[q, b] += WT_k[f,q].T @ xT_k[f,b], bias
    folded in as a K=1 accumulate matmul (bias[q] x ones[b]).
  - sumexp over q: ones[128,1].T @ exp[q,b] matmul (PE reduces over
    partitions), lse = Ln(sumexp) on ScalarE.
  - gather+logsoftmax in one PSUM group:
        psum_out[b, c] = linT[q,b].T @ OH[q,c] + lse[b].T @ (-1)[c]
    (the K=1 lse matmul also transposes lse into the partition dim).
    OH_p[q, c] = (q == part_idx[p,c]) is built per-p on DVE with an
    is_equal against a partition iota.

This walrus build only accepts ONE sync-wait command per compute/DMA
instruction, which dictates most of the structure:
  - x|W share one DMA per k-tile ("xw"); bias|ones share one DMA
    ("bo"), so each matmul joins on a single semaphore.
  - every SBUF tile is used exactly once (fresh slot) -> no
    WAR/WAW slot-release waits anywhere.
  - idx and iota for partitioning p are DMAd back-to-back so the
    SWDGE round-robin lands them on the same queue semaphore; the
    is_equal TT then joins on that one sem.
  - PSUM->SBUF result copies all run on DVE into per-(p-pair,bt)
    group tiles; each output DMA (on the ACT HWDGE) is preceded by a
    tiny ACT "observer" op that absorbs the DVE producer wait, so the
    DMA itself only carries its queue-predecessor wait.
  - bf16 for x/W (also halves their HBM traffic); the gather path is
    float32r (full-rate PE fp32).
"""

import numpy as np

B, P, Q, C, F = 256, 64, 128, 1000, 2048
NCORES = 8
PL = P // NCORES          # partitionings per core
KT = F // 128             # contraction tiles
BT = B // 128             # batch tiles for the gather matmul
C_CHUNKS = [(0, 512), (512, C - 512)]

MAIN_BF16 = True          # store/stream x,W as bf16 and matmul in bf16
GATHER_R = True           # gather/lse/sumexp matmul operands in float32r


def _build_nc():
    import concourse.bass as bass
    import concourse.tile as tile
    from concourse import mybir
    from contextlib import ExitStack

    DT = mybir.dt.float32
    HT = mybir.dt.float16
    MDT = mybir.dt.bfloat16 if MAIN_BF16 else mybir.dt.float32r
    # bf16 gather operands: 2-byte stationary loads keep the PE at full
    # rate (fp32r 4-byte weight loads measured ~2x slower per matmul)
    GDT = mybir.dt.bfloat16

    nc = bass.Bass()
    xw_d = nc.declare_dram_parameter("xw", [KT, 128, B + PL * Q], MDT,
                                     isOutput=False)
    bo_d = nc.declare_dram_parameter("bo", [1, PL * Q + B], MDT,
                                     isOutput=False)
    # idxq[q, p, :C] = part_idx[p, :] (same on every partition row) and
    # idxq[q, p, C] = q — idx and iota in ONE tensor/DMA, so the
    # is_equal TT joins on a single DMA semaphore
    idx_d = nc.declare_dram_parameter("idxq", [Q, PL, C + 1], HT,
                                      isOutput=False)
    out_d = nc.declare_dram_parameter("out", [B, PL, C], DT, isOutput=True)

    with ExitStack() as ctx:
        tc = ctx.enter_context(tile.TileContext(nc))
        singles = ctx.enter_context(tc.tile_pool(name="singles", bufs=1))
        ps_lin = ctx.enter_context(
            tc.tile_pool(name="ps_lin", bufs=2, space=bass.MemorySpace.PSUM))
        ps_sum = ctx.enter_context(
            tc.tile_pool(name="ps_sum", bufs=2, space=bass.MemorySpace.PSUM))
        ps_out = ctx.enter_context(
            tc.tile_pool(name="ps_out", bufs=4, space=bass.MemorySpace.PSUM))

        def fresh(shape, dtype, tag):
            return singles.tile(shape, dtype, tag=tag, name=tag)

        # ---- static tiles (all fresh, single-use) -------------------
        xwk = []
        for k in range(KT):
            t = fresh([128, B + PL * Q], MDT, f"xwk{k}")
            nc.sync.dma_start(out=t[:], in_=xw_d[k])
            xwk.append(t)
        bo_sb = fresh([1, PL * Q + B], MDT, "bo")
        nc.sync.dma_start(out=bo_sb[:], in_=bo_d[:])
        idx_sb = fresh([128, PL, C + 1], HT, "idxq")
        nc.sync.dma_start(out=idx_sb[:], in_=idx_d[:])

        # ACT-produced constants so the ACT-side matmuls join on ACT
        ones_col = fresh([128, 1], GDT, "ones")
        nc.scalar.activation(out=ones_col[:], in_=xwk[0][:, 0:1],
                             func=mybir.ActivationFunctionType.Copy,
                             bias=1.0, scale=0.0)
        negones_sb = fresh([1, 512], GDT, "negones")
        nc.scalar.activation(out=negones_sb[:], in_=bo_sb[0:1, 0:512],
                             func=mybir.ActivationFunctionType.Copy,
                             bias=-1.0, scale=0.0)

        obs_scratch = fresh([1, 4 * PL], DT, "obs")

        # ---- per-partitioning pipeline ------------------------------
        og_tiles = {}
        n_obs = 0
        for p in range(PL):
            psum_lin = ps_lin.tile([128, B], DT)
            # bias: K=1 matmul bias[q] x ones[b] opens the accumulation
            nc.tensor.matmul(
                psum_lin[:],
                bo_sb[:, p * Q:(p + 1) * Q],
                bo_sb[:, PL * Q:],
                start=True, stop=False)
            for k in range(KT):
                nc.tensor.matmul(
                    psum_lin[:],
                    xwk[k][:, B + p * Q:B + (p + 1) * Q],
                    xwk[k][:, :B],
                    start=False,
                    stop=(k == KT - 1),
                )

            # one-hot build for this p on DVE (single DMA sem join)
            oh_p = fresh([128, C], GDT, f"oh{p}")
            nc.vector.tensor_tensor(
                out=oh_p[:],
                in0=idx_sb[:, p, :C],
                in1=idx_sb[:, p, C:C + 1].broadcast_to((128, C)),
                op=mybir.AluOpType.is_equal,
            )

            linT = fresh([128, B], GDT, f"lin{p}")
            nc.vector.tensor_copy(linT[:], psum_lin[:])
            expT = fresh([128, B], GDT, f"exp{p}")
            nc.scalar.activation(
                out=expT[:], in_=linT[:],
                func=mybir.ActivationFunctionType.Exp)

            psum_sum = ps_sum.tile([1, B], DT)
            nc.tensor.matmul(
                psum_sum[:], ones_col[:], expT[:],
                start=True, stop=True)
            lse = fresh([1, B], GDT, f"lse{p}")
            nc.scalar.activation(
                out=lse[:], in_=psum_sum[:],
                func=mybir.ActivationFunctionType.Ln)

            pair = p // 2
            for bt in range(BT):
                bsl = slice(bt * 128, (bt + 1) * 128)
                if p % 2 == 0:
                    og_new = fresh([128, 2, C], DT, f"og{pair}_{bt}")
                    og_tiles[(pair, bt)] = og_new
                og = og_tiles[(pair, bt)]
                last_copy = None
                for (c0, cw) in C_CHUNKS:
                    psum_out = ps_out.tile([128, 512], DT)
                    nc.tensor.matmul(
                        psum_out[:, :cw],
                        linT[:, bsl],
                        oh_p[:, c0:c0 + cw],
                        start=True, stop=False)
                    nc.tensor.matmul(
                        psum_out[:, :cw],
                        lse[:, bsl],
                        negones_sb[:, :cw],
                        start=False, stop=True)
                    last_copy = nc.vector.tensor_copy(
                        og[:, p % 2, c0:c0 + cw], psum_out[:, :cw])
                if p % 2 == 1:
                    # ACT observer absorbs the DVE producer wait; the
                    # DMA then only carries its queue-predecessor wait
                    obs = nc.scalar.activation(
                        out=obs_scratch[0:1, n_obs:n_obs + 1],
                        in_=og[0:1, 1, C - 1:C],
                        func=mybir.ActivationFunctionType.Copy,
                        bias=0.0, scale=1.0)
                    n_obs += 1
                    dma = nc.scalar.dma_start(
                        out=out_d[bsl, p - 1:p + 1, :],
                        in_=og[:])
                    tile.add_dep_helper(dma.ins, obs.ins, sync=False,
                                        reason="dma after observer")

    _install_drain_split(nc)
    return nc


def _install_drain_split(nc, chunk=1):
    """The kernel-tail Drain waits on every live semaphore (~11), but
    this walrus build's CTRL_NO encoding fits only a couple of sync
    commands.  Splitting the drain into a chain of drains, each
    carrying `chunk` waits, is semantically identical (sequential SP
    sem waits).  Patch at serialization time so every consumer of
    nc.to_json_bytes() sees the legal form."""
    import copy
    import json

    orig = nc.to_json_bytes

    def patched():
        m = json.loads(orig())
        for fn in m["functions"]:
            for bb in fn["blocks"]:
                out = []
                for inst in bb["instructions"]:
                    si = inst.get("sync_info")
                    if (inst.get("opcode") == "Drain" and si
                            and si.get("on_wait")
                            and len(si["on_wait"]) > chunk):
                        waits = si["on_wait"]
                        head, keep = waits[:-chunk], waits[-chunk:]
                        for j in range(0, len(head), chunk):
                            clone = copy.deepcopy(inst)
                            clone["name"] = f"{inst['name']}-ds{j}"
                            clone["sync_info"] = {
                                "on_wait": head[j:j + chunk],
                                "on_update": [],
                            }
                            out.append(clone)
                        si["on_wait"] = keep
                    out.append(inst)
                bb["instructions"] = out
        return json.dumps(m).encode()

    nc.to_json_bytes = patched


def _host_inputs(x, W, b, part_idx):
    """Build the 8 per-core input maps."""
    import ml_dtypes

    mm_np = ml_dtypes.bfloat16 if MAIN_BF16 else np.float32
    xT = x.T.reshape(KT, 128, B).astype(mm_np)                # [KT,128,B]
    in_maps = []
    for i in range(NCORES):
        sl = slice(i * PL, (i + 1) * PL)
        WT = W[sl].transpose(2, 0, 1).reshape(
            KT, 128, PL * Q).astype(mm_np)                    # [KT,128,PL*Q]
        xw = np.empty((KT, 128, B + PL * Q), dtype=mm_np)
        xw[:, :, :B] = xT
        xw[:, :, B:] = WT
        bo = np.empty((1, PL * Q + B), dtype=mm_np)
        bo[0, :PL * Q] = b[sl].reshape(-1)
        bo[0, PL * Q:] = 1.0
        idxq = np.empty((Q, PL, C + 1), dtype=np.float16)
        idxq[:, :, :C] = part_idx[sl].astype(np.float16)[None, :, :]
        idxq[:, :, C] = np.arange(Q, dtype=np.float16)[:, None]
        in_maps.append({"xw": xw, "bo": bo, "idxq": idxq})
    return in_maps


def kernel(x, W, b, part_idx, _trace=False):
    from concourse.bass_utils import run_bass_kernel_spmd

    x = np.asarray(x, dtype=np.float32)
    W = np.asarray(W, dtype=np.float32)
    b = np.asarray(b, dtype=np.float32)
    part_idx = np.asarray(part_idx)

    nc = _build_nc()
    in_maps = _host_inputs(x, W, b, part_idx)
    res = run_bass_kernel_spmd(nc, in_maps, list(range(NCORES)),
                               trace=_trace)
    out = np.concatenate([r["out"] for r in res.results], axis=1)
    if _trace:
        return out, res
    return out



# revision 10
# speedup vs baseline: 1.4581x; 1.4581x over previous
"""Trainium2 Bass kernel for nn_CombinatorialClassifier.

Computation (reference):
    logits = einsum('bf,pqf->bpq', x, W) + b        # [B,P,Q]
    logp   = log_softmax(logits, axis=2)            # [B,P,Q]
    out    = take_along_axis(logp, part_idx, 2)     # [B,P,C]

Shapes: B=256, P=64, Q=128, C=1000, F=2048.

Sharding: expert-parallel over P across 8 cores (8 partitionings per
core).  Each core reads the full x and its W/b/part_idx slice and
writes its disjoint [B, 8, C] slice of the output.  No collectives.

Per-core dataflow (PSUM orientation [q, b] for the linear part):
  - W arrives per-p (one DMA per partitioning, x combined with W0) so
    p=0's matmuls start after ~1.7MB of DMA instead of the full 7MB.
  - lin group: bias K=1 matmul opens, 16 k-tile matmuls accumulate;
    after exp/sumexp/ln a final K=1 matmul (negones[q] x lse[b]) adds
    -lse[b] to every element, so psum_lin holds log-softmax directly.
  - gather: psum_out[b, c] = logpT[q,b].T @ OH[q,c] with the one-hot
    OH built on the HOST (exact 0/1) and shipped as an input; the
    PSUM->SBUF drains are then PLAIN dtype-cast copies (single dep),
    alternating DVE / ACT per p-pair.
  - output staged in SBUF fp16 (halves out traffic); host casts back
    to fp32.

This walrus build fits only ONE sync-wait per instruction.  Instead of
contorting the dataflow, _install_wait_split post-processes the
serialized IR: every instruction with N>1 waits keeps one and gets
N-1 standalone single-wait EventSemaphore instructions immediately
before it on the same engine/queue — semantically identical.
"""

import numpy as np

B, P, Q, C, F = 256, 64, 128, 1000, 2048
NCORES = 8
PL = P // NCORES          # partitionings per core
KT = F // 128             # contraction tiles
BT = B // 128             # batch tiles for the gather matmul
C_CHUNKS = [(0, 512), (512, C - 512)]
KW = B + Q                # combined x|w0 column block per k-tile

# dtypes (mybir names) for the big streamed operands
X_DT = "float16"          # x (+W0) (x is moving in the main matmul)
W_DT = "float16"          # W p>=1 (stationary in main matmul)
OH_DT = "float16"         # one-hot gather matrix (moving in gather)


def _np_dt(name):
    import ml_dtypes
    return {
        "float16": np.float16,
        "bfloat16": ml_dtypes.bfloat16,
        "float8e4": ml_dtypes.float8_e4m3fn,
        "float32": np.float32,
    }[name]


def _build_nc():
    import concourse.bass as bass
    import concourse.tile as tile
    from concourse import mybir
    from contextlib import ExitStack

    DT = mybir.dt.float32
    HT = mybir.dt.float16
    XDT = getattr(mybir.dt, X_DT)
    WDT = getattr(mybir.dt, W_DT)
    OHDT = getattr(mybir.dt, OH_DT)
    AF = mybir.ActivationFunctionType

    nc = bass.Bass()
    const_d = nc.declare_dram_parameter(
        "const", [1, PL * Q + B + 128], HT, isOutput=False)
    xw0_d = nc.declare_dram_parameter("xw0", [128, KT * KW], XDT,
                                      isOutput=False)
    w_d = nc.declare_dram_parameter("win", [PL - 1, 128, KT * Q], WDT,
                                    isOutput=False)
    oh_d = nc.declare_dram_parameter("ohin", [PL, 128, C], OHDT,
                                     isOutput=False)
    out_d = nc.declare_dram_parameter("out", [B, PL, C], HT, isOutput=True)

    OFF_BIAS = 0
    OFF_ONES = PL * Q
    OFF_NEG = PL * Q + B

    with ExitStack() as ctx:
        tc = ctx.enter_context(tile.TileContext(nc))
        singles = ctx.enter_context(tc.tile_pool(name="singles", bufs=1))
        ps_lin = ctx.enter_context(
            tc.tile_pool(name="ps_lin", bufs=3, space=bass.MemorySpace.PSUM))
        ps_sum = ctx.enter_context(
            tc.tile_pool(name="ps_sum", bufs=1, space=bass.MemorySpace.PSUM))
        ps_out = ctx.enter_context(
            tc.tile_pool(name="ps_out", bufs=2, space=bass.MemorySpace.PSUM))

        def fresh(shape, dtype, tag):
            return singles.tile(shape, dtype, tag=tag, name=tag)

        # ---- input DMAs (SP queue, in order) -------------------------
        const_sb = fresh([1, PL * Q + B + 128], HT, "const")
        nc.sync.dma_start(out=const_sb[:], in_=const_d[:])
        xw0_sb = fresh([128, KT * KW], XDT, "xw0")
        nc.sync.dma_start(out=xw0_sb[:], in_=xw0_d[:])
        w_sb = [xw0_sb]
        oh_sb = []
        for p in range(PL):
            ot = fresh([128, C], OHDT, f"oh{p}")
            nc.sync.dma_start(out=ot[:], in_=oh_d[p])
            oh_sb.append(ot)
            if p + 1 < PL:
                wt = fresh([128, KT * Q], WDT, f"w{p + 1}")
                nc.sync.dma_start(out=wt[:], in_=w_d[p])
                w_sb.append(wt)

        def w_slice(p, k):
            if p == 0:
                return xw0_sb[:, k * KW + B:(k + 1) * KW]
            return w_sb[p][:, k * Q:(k + 1) * Q]

        def x_slice(k):
            return xw0_sb[:, k * KW:k * KW + B]

        # ones column for the sumexp matmuls (ACT-made, dep on xw0 DMA)
        ones_col = fresh([128, 1], HT, "ones_col")
        nc.scalar.activation(out=ones_col[:], in_=xw0_sb[:, 0:1],
                             func=AF.Copy, bias=1.0, scale=0.0)

        # ---- per-partitioning pipeline ------------------------------
        og_tiles = {}
        for p in range(PL):
            pair = p // 2
            on_dve = (pair % 2 == 0)

            psum_lin = ps_lin.tile([128, B], DT)
            nc.tensor.matmul(
                psum_lin[:],
                const_sb[:, OFF_BIAS + p * Q:OFF_BIAS + (p + 1) * Q],
                const_sb[:, OFF_ONES:OFF_ONES + B],
                start=True, stop=False)
            for k in range(KT):
                nc.tensor.matmul(
                    psum_lin[:], w_slice(p, k), x_slice(k),
                    start=False, stop=(k == KT - 1))

            expT = fresh([128, B], HT, f"exp{p}")
            nc.scalar.activation(out=expT[:], in_=psum_lin[:], func=AF.Exp)

            psum_sum = ps_sum.tile([1, B], DT)
            nc.tensor.matmul(
                psum_sum[:], ones_col[:], expT[:],
                start=True, stop=True)
            lse = fresh([1, B], HT, f"lse{p}")
            nc.scalar.activation(out=lse[:], in_=psum_sum[:], func=AF.Ln)

            # -lse folded into the linear psum: psum[q,b] += (-1)*lse[b]
            nc.tensor.matmul(
                psum_lin[:],
                const_sb[:, OFF_NEG:OFF_NEG + 128],
                lse[:],
                start=False, stop=True, skip_group_check=True)

            linT = fresh([128, B], HT, f"lin{p}")
            nc.vector.tensor_copy(linT[:], psum_lin[:])

            for bt in range(BT):
                bsl = slice(bt * 128, (bt + 1) * 128)
                if p % 2 == 0:
                    og_tiles[(pair, bt)] = fresh([128, 2, C], HT,
                                                 f"og{pair}_{bt}")
                og = og_tiles[(pair, bt)]
                psum_out = ps_out.tile([128, 1024], DT)
                for (c0, cw) in C_CHUNKS:
                    nc.tensor.matmul(
                        psum_out[:, c0:c0 + cw],
                        linT[:, bsl],
                        oh_sb[p][:, c0:c0 + cw],
                        start=True, stop=True)
                if on_dve:
                    nc.vector.tensor_copy(og[:, p % 2, :], psum_out[:, :C])
                else:
                    nc.scalar.activation(out=og[:, p % 2, :],
                                         in_=psum_out[:, :C], func=AF.Copy)
                if p % 2 == 1:
                    nc.sync.dma_start(
                        out=out_d[bsl, p - 1:p + 1, :],
                        in_=og[:])

    _install_wait_split(nc)
    return nc


def _install_wait_split(nc):
    """Walrus fits ONE sync-wait per instruction.  For every instruction
    carrying N>1 waits, keep the last and emit N-1 standalone
    EventSemaphore instructions (same engine, one wait each) before it.
    Engines execute their stream in order, so this is semantically
    identical.  Applied at serialization time so every consumer of
    nc.to_json_bytes() sees the legal form."""
    import json

    orig = nc.to_json_bytes

    def patched():
        m = json.loads(orig())
        n_split = 0
        for fn in m["functions"]:
            for bb in fn["blocks"]:
                out = []
                for inst in bb["instructions"]:
                    si = inst.get("sync_info")
                    if si and si.get("on_wait") and len(si["on_wait"]) > 1:
                        waits = si["on_wait"]
                        head, keep = waits[:-1], waits[-1:]
                        for j, w in enumerate(head):
                            out.append({
                                "debug": inst.get("debug", 0),
                                "engine": inst["engine"],
                                "ins": [],
                                "name": f"{inst['name']}-ws{j}",
                                "opcode": "EventSemaphore",
                                "outs": [],
                                "sync_info": {
                                    "on_update": [],
                                    "on_wait": [w],
                                },
                            })
                            n_split += 1
                        si["on_wait"] = keep
                    out.append(inst)
                bb["instructions"] = out
        return json.dumps(m).encode()

    nc.to_json_bytes = patched


def _host_inputs(x, W, b, part_idx):
    """Build the 8 per-core input maps."""
    x_np = _np_dt(X_DT)
    w_np = _np_dt(W_DT)
    oh_np = _np_dt(OH_DT)

    xT = np.ascontiguousarray(x.T.reshape(KT, 128, B))     # [KT,128,B]
    qs = np.arange(Q)
    in_maps = []
    for i in range(NCORES):
        sl = slice(i * PL, (i + 1) * PL)
        Wc = W[sl]                                     # [PL, Q, F]
        wT = Wc.transpose(2, 0, 1).reshape(KT, 128, PL, Q)  # [KT,128,PL,Q]
        xw0 = np.empty((128, KT, KW), dtype=x_np)
        xw0[:, :, :B] = xT.transpose(1, 0, 2)
        xw0[:, :, B:] = wT[:, :, 0, :].transpose(1, 0, 2)
        wh = np.ascontiguousarray(
            wT[:, :, 1:, :].transpose(2, 1, 0, 3)
              .reshape(PL - 1, 128, KT * Q)).astype(w_np)
        idx = part_idx[sl]                             # [PL, C]
        oh = (idx[:, None, :] == qs[None, :, None]).astype(oh_np)
        const = np.zeros((1, PL * Q + B + 128), dtype=np.float16)
        const[0, :PL * Q] = b[sl].reshape(-1).astype(np.float16)
        const[0, PL * Q:PL * Q + B] = 1.0
        const[0, PL * Q + B:] = -1.0
        in_maps.append({
            "const": const,
            "xw0": xw0.reshape(128, KT * KW),
            "win": wh,
            "ohin": oh,
        })
    return in_maps


def kernel(x, W, b, part_idx, _trace=False):
    from concourse.bass_utils import run_bass_kernel_spmd

    x = np.asarray(x, dtype=np.float32)
    W = np.asarray(W, dtype=np.float32)
    b = np.asarray(b, dtype=np.float32)
    part_idx = np.asarray(part_idx)

    nc = _build_nc()
    in_maps = _host_inputs(x, W, b, part_idx)
    res = run_bass_kernel_spmd(nc, in_maps, list(range(NCORES)),
                               trace=_trace)
    out = np.concatenate(
        [r["out"].astype(np.float32) for r in res.results], axis=1)
    if _trace:
        return out, res
    return out
